# revision 4
# baseline (speedup 1.0000x reference)
"""Trainium2 Bass kernel for nn_Conv2DLayer_16011638080159.

Math: out = C * (x @ weight.sum(0))   with x [524288, 512], weight [9, 512].
Equivalent to a row-wise dot product of x with w_eff = C * weight.sum(0).

Strategy (pure data parallel, per sharding hint):
  - Shard x along the batch axis across 8 NeuronCores (65536 rows each).
  - Host-side prep: fold the tiny K=9 weight sum and the C scale into a
    single [C] vector, replicated to a [128, R*C] SBUF-ready constant.
  - Per core: stream x tiles [128 partitions, R rows x 512] from HBM,
    multiply by the replicated weight on the Vector engine (fp32 TT, 1x),
    and accumulate each row's 512-element segment on the Scalar engine via
    ACTIVATE(Copy, accum_out) -- a free-dim sum at 1 elem/cycle -- so both
    compute engines stay under the ~373 us/core HBM streaming floor.
  - Row mapping: shard row (p*512 + t*R + r) sits at partition p, tile t,
    slot r, so the per-core result tile [128, 512] is exactly the row-major
    view of the per-core output [65536]; one contiguous DMA writes it out.
"""

import numpy as np

import concourse.bacc as bacc
import concourse.bass as bass
import concourse.tile as tile
from concourse import mybir
from concourse.bass_utils import run_bass_kernel_spmd

B = 524288        # total rows
C = 512           # row length
N_CORES = 8
BS = B // N_CORES  # 65536 rows per core
P = 128            # SBUF partitions
RPP = BS // P      # 512 rows per partition
R = 8              # rows per partition per tile
F = R * C          # 4096 free elems per tile
NT = RPP // R      # 64 tiles per core

_NC_CACHE = None
LAST_RESULT = None  # BassKernelResults of the most recent run (for profiling)


def _build() -> bass.Bass:
    # Bacc (not raw Bass): its compile() pass splits multi-sem waits into
    # EventSemaphore instructions -- the TRN2 ISA allows only 1 wait/inst.
    nc = bacc.Bacc(None, target_bir_lowering=False, debug=False)
    x = nc.dram_tensor("x", [BS, C], mybir.dt.float32, kind="ExternalInput")
    w = nc.dram_tensor("w", [P, F], mybir.dt.float32, kind="ExternalInput")
    out = nc.dram_tensor("out", [BS], mybir.dt.float32, kind="ExternalOutput")

    # shard row (p*RPP + t*R + r) -> partition p, tile t, free slot (r, c)
    xv = x.rearrange("(p t r) c -> t p (r c)", p=P, t=NT, r=R)
    ov = out.rearrange("(p f) -> p f", p=P)

    with tile.TileContext(nc) as tc:
        with (
            tc.tile_pool(name="const", bufs=1) as cpool,
            tc.tile_pool(name="xs", bufs=4) as xs,
            tc.tile_pool(name="ys", bufs=3) as ys,
            tc.tile_pool(name="scr", bufs=2) as scr,
            tc.tile_pool(name="res", bufs=1) as res,
        ):
            w_t = cpool.tile([P, F], mybir.dt.float32)
            nc.sync.dma_start(out=w_t[:], in_=w[:, :])
            o_t = res.tile([P, RPP], mybir.dt.float32)
            for t in range(NT):
                x_t = xs.tile([P, F], mybir.dt.float32)
                nc.sync.dma_start(out=x_t[:], in_=xv[t])
                y_t = ys.tile([P, F], mybir.dt.float32)
                nc.vector.tensor_mul(y_t[:], x_t[:], w_t[:])
                for r in range(R):
                    s_t = scr.tile([P, C], mybir.dt.float32)
                    nc.scalar.activation(
                        out=s_t[:],
                        in_=y_t[:, r * C:(r + 1) * C],
                        func=mybir.ActivationFunctionType.Copy,
                        accum_out=o_t[:, t * R + r: t * R + r + 1],
                    )
            nc.sync.dma_start(out=ov, in_=o_t[:])
    nc.finalize()
    return nc


def kernel(x: np.ndarray, weight: np.ndarray) -> np.ndarray:
    global _NC_CACHE, LAST_RESULT
    x = np.ascontiguousarray(np.asarray(x), dtype=np.float32)
    weight = np.asarray(weight, dtype=np.float32)

    w_eff = (C * weight.sum(axis=0)).astype(np.float32)   # [C]
    w_rep = np.ascontiguousarray(np.tile(w_eff, (P, R)))  # [P, F]

    if _NC_CACHE is None:
        _NC_CACHE = _build()

    in_maps = [
        {"x": x[i * BS:(i + 1) * BS], "w": w_rep} for i in range(N_CORES)
    ]
    LAST_RESULT = run_bass_kernel_spmd(
        _NC_CACHE, in_maps, core_ids=list(range(N_CORES))
    )
    return np.concatenate([r["out"] for r in LAST_RESULT.results])
